# revision 25
# baseline (speedup 1.0000x reference)
"""Trainium2 Bass kernel for nn_EpiNN_aaindex (pairwise-MLP GNN reduction).

Math (per batch b):
  x1 = emb@tw + tb                              (computed on HOST, f32)
  X[i,d] = emb[i*64+d] * tw[i*64+d]             (L=256, D=64; on HOST, bf16)
  s_ij = MLP(concat[(x_i+x_j)/2, |x_i-x_j|])    (128->64->16->1, LeakyReLU)
  out_b = x1 + scale * sum_{i<j} s_ij

Strategy: 8 cores, 4 batches/core (data parallel over B=32).
Exact upper-triangle enumeration via cyclic offsets o=1..128:
pairs (i, (i+o) mod 256) for o=1..127 cover each unordered pair once;
o=128's second half (i>=128) is excluded on-device via a split L2 matmul.

Device layouts (per batch, SBUF, bf16; X2T = X.T [64 d, 256 i], from host):
  XU  [128, 256]: both lane halves = X2T
  XSN [128, 512]: lanes 0:64 = X2T|X2T, lanes 64:128 = -(X2T|X2T)
Main loop: flat list of 32 (batch, p) pairs, 16 offsets each:
o = 8p+1+b (+64j), j in {0,1}, b in 0..7 (half A: b 0..3, B: b 4..7).
  M [128, 16, 256] = XU - XSN[:, win] (ONE DVE op; overlapping windows via
    custom AP [[64,2],[1,8],[1,256]] starting at col 8p+1):
    block m=8j+b: lanes 0:64 = x_i - x_{i+o}, lanes 64:128 = x_i + x_{i+o}
  abs on lanes 0:64 (u16 bitmask), then per half 4 512-free 128-contraction
  matmuls, stationary [w1b.T ; 0.5*w1a.T] -> P1 [128, 1024]
  (lanes 0:64 = j=0 offsets, 64:128 = j=1);
  act1 Lrelu+b1 -> H1 bf16 [128, 1024];
  L2: 4 matmuls, stationary = block-diag w2.T pairs -> P2D [128, 512]
  (half A cols 0:256, B 256:512; lane r: b=r//32, j=(r%32)//16, f=r%16);
  act2 Lrelu+b2 -> HJ, then DVE tensor_reduce -> ACC[:, p].
L2 and act2 are emitted one pair behind L1/act1 (software pipelining) so
the PE and Act engines never stall on same-pair dependencies.
o=128 (p=7, B, b=7, j=1): its L2 only covers i<128; cols 384:512 of lanes
112:128 are zero-filled, adding exactly 128*lrelu(b2) to ACC[112:128, 7]
(host subtracts).

Final combine on host: out = x1 + scale*(w3 . R + 32640*b3).
"""
import numpy as np

L, D = 256, 64
B_PER_CORE = 4
N_CORES = 8
NPAIRS = 32640  # 256*255/2

_CACHE = {}
import os as _os
N_BATCH = int(_os.environ.get("EPINN_BATCH", str(B_PER_CORE)))
N_RUN_CORES = int(_os.environ.get("EPINN_CORES", str(N_CORES)))


def _build_program():
    import concourse.bacc as bacc
    import concourse.mybir as mybir
    import concourse.tile as tile
    from concourse.bass import AP
    from contextlib import ExitStack

    f32 = mybir.dt.float32
    bf16 = mybir.dt.bfloat16
    u16 = mybir.dt.uint16
    AF = mybir.ActivationFunctionType
    ALU = mybir.AluOpType

    nc = bacc.Bacc("TRN2", target_bir_lowering=False, debug=False,
                   num_devices=N_CORES)

    # ---- DRAM parameters (per core) ----
    # embx[b] = [2, 64, 256]: [0] = X2T (bf16), [1] = -X2T
    embx_d = nc.declare_dram_parameter("embx", [B_PER_CORE, 2, D, L], bf16,
                                       isOutput=False)
    w1fd_d = nc.declare_dram_parameter("w1fd", [128, 128], bf16, isOutput=False)
    w2d4_d = nc.declare_dram_parameter("w2d4", [128, 128], bf16, isOutput=False)
    w2t_d = nc.declare_dram_parameter("w2t2", [128, 64], bf16, isOutput=False)
    b1s_d = nc.declare_dram_parameter("b1s", [128, 1], f32, isOutput=False)
    b2s_d = nc.declare_dram_parameter("b2s", [128, 1], f32, isOutput=False)

    acc_o = nc.declare_dram_parameter("acc_o", [B_PER_CORE, 128, 8], f32,
                                      isOutput=True)

    with tile.TileContext(nc) as tc, ExitStack() as ctx:
        cpool = ctx.enter_context(tc.tile_pool(name="consts", bufs=1))
        ppool = ctx.enter_context(tc.tile_pool(name="persist", bufs=1))
        mpool = ctx.enter_context(tc.tile_pool(name="mbufs", bufs=3))
        hpool = ctx.enter_context(tc.tile_pool(name="hbufs", bufs=4))
        jpool = ctx.enter_context(tc.tile_pool(name="junk", bufs=2))
        pp1 = ctx.enter_context(tc.tile_pool(name="p1", bufs=3, space="PSUM"))
        pp2 = ctx.enter_context(tc.tile_pool(name="p2", bufs=2, space="PSUM"))

        # ---- static weights / consts ----
        W1FD = cpool.tile([128, 128], bf16)
        W2D4 = cpool.tile([128, 128], bf16)
        W2T = cpool.tile([128, 64], bf16)
        B1S = cpool.tile([128, 1], f32)
        B2S = cpool.tile([128, 1], f32)

        nc.sync.dma_start(W1FD[:], w1fd_d[:])
        nc.sync.dma_start(W2D4[:], w2d4_d[:])
        nc.sync.dma_start(W2T[:], w2t_d[:])
        nc.sync.dma_start(B1S[:], b1s_d[:])
        nc.sync.dma_start(B2S[:], b2s_d[:])

        XUs = [None] * N_BATCH
        XSNs = [None] * N_BATCH
        ACCs = [None] * N_BATCH

        def emit_setup(b):
            XU = ppool.tile([128, 256], bf16, name=f"xu{b}")
            XSN = ppool.tile([128, 512], bf16, name=f"xsn{b}")
            src = embx_d[b].rearrange("g d f -> (g d) f")  # [128, 256]
            nc.sync.dma_start(XSN[:, 0:256], src)
            nc.sync.dma_start(XSN[:, 256:512], src)
            nc.sync.dma_start(XU[0:64, :], embx_d[b, 0])
            nc.sync.dma_start(XU[64:128, :], embx_d[b, 0])
            ACC = ppool.tile([128, 8], f32, name=f"acc{b}")
            nc.gpsimd.memset(ACC[:], 0.0)
            XUs[b] = XU
            XSNs[b] = XSN
            ACCs[b] = ACC

        def emit_front(b, p):
            """sub/abs + L1 + act1 for pair (b, p); returns H1 tiles."""
            XU, XSN = XUs[b], XSNs[b]
            pstride = XU[:].ap.copy()[0][0]
            XU_B = AP(XU.tensor, XU.offset,
                      [[pstride, 128], [0, 2], [0, 8], [1, 256]])
            xsn_pstride = XSN[:].ap.copy()[0][0]

            o0 = 8 * p + 1
            M = mpool.tile([128, 16, 256], bf16, tag="m")
            MW = M[:].rearrange("p (a b) f -> p a b f", a=2)
            XSN_W = AP(XSN.tensor, XSN.offset + o0,
                       [[xsn_pstride, 128], [64, 2], [1, 8], [1, 256]])
            nc.vector.tensor_tensor(out=MW, in0=XU_B, in1=XSN_W,
                                    op=ALU.subtract)
            nc.vector.tensor_scalar(
                out=M[0:64, :, :].bitcast(u16),
                in0=M[0:64, :, :].bitcast(u16),
                scalar1=0x7FFF, scalar2=None, op0=ALU.bitwise_and)

            H1s = []
            for h in (0, 1):  # half A/B
                P1 = pp1.tile([128, 1024], f32, tag="p1")
                for j in (0, 1):
                    lhs = W1FD[:, 64 * j:64 * j + 64]
                    for bb in (0, 2):
                        m0 = 8 * j + 4 * h + bb
                        nc.tensor.matmul(
                            P1[64 * j:64 * j + 64, 256 * bb:256 * bb + 512],
                            lhs, M[:, m0:m0 + 2, :],
                            start=True, stop=True, skip_group_check=True)
                H1 = hpool.tile([128, 1024], bf16, tag="h1")
                nc.scalar.activation(H1[:], P1[:], AF.Lrelu, bias=B1S[:],
                                     scale=1.0, alpha=0.01)
                H1s.append(H1)
            return H1s

        def emit_back(b, p, H1s):
            """L2 + act2 + accum for pair (b, p)."""
            ACC = ACCs[b]
            P2D = pp2.tile([128, 512], f32, tag="p2")
            for h in (0, 1):
                H1 = H1s[h]
                for bb in range(4):
                    if p == 7 and h == 1 and bb == 3:
                        nc.tensor.matmul(
                            P2D[96:128, 256:512], W2T[:, 0:32],
                            H1[:, 768:1024],
                            start=True, stop=False, skip_group_check=True,
                            tile_position=(0, 96))
                        nc.tensor.matmul(
                            P2D[96:128, 256:384], W2T[:, 32:64],
                            H1[:, 768:896],
                            start=False, stop=True, skip_group_check=True,
                            tile_position=(0, 96))
                    else:
                        nc.tensor.matmul(
                            P2D[32 * bb:32 * bb + 32, 256 * h:256 * h + 256],
                            W2D4[:, 32 * bb:32 * bb + 32],
                            H1[:, 256 * bb:256 * bb + 256],
                            start=True, stop=True, skip_group_check=True,
                            tile_position=(0, 32 * bb))

            HJ = jpool.tile([128, 512], bf16, tag="hj")
            nc.scalar.activation(HJ[:], P2D[:], AF.Lrelu, bias=B2S[:],
                                 scale=1.0, alpha=0.01)
            nc.vector.tensor_reduce(out=ACC[:, p:p + 1], in_=HJ[:],
                                    op=ALU.add, axis=mybir.AxisListType.X)

        # ---- software-pipelined emission over the flat pair list ----
        emit_setup(0)
        pairs = [(b, p) for b in range(N_BATCH) for p in range(8)]
        prev = None
        for k, (b, p) in enumerate(pairs):
            H1s = emit_front(b, p)
            if prev is not None:
                pb, pp, pH = prev
                emit_back(pb, pp, pH)
                if pp == 7:
                    nc.sync.dma_start(acc_o[pb], ACCs[pb][:])
            prev = (b, p, H1s)
            if p == 0 and b + 1 < N_BATCH:
                emit_setup(b + 1)
        pb, pp, pH = prev
        emit_back(pb, pp, pH)
        nc.sync.dma_start(acc_o[pb], ACCs[pb][:])

    nc.compile()
    return nc


def _get_program():
    key = ("prog", N_BATCH)
    if key not in _CACHE:
        _CACHE[key] = _build_program()
    return _CACHE[key]


def _get_runner():
    """Build (once) a cached jitted SPMD executable for the program."""
    key = ("runner", N_BATCH, N_RUN_CORES)
    if key in _CACHE:
        return _CACHE[key]
    import jax
    import numpy as _np
    import concourse.mybir as mybir
    from jax.sharding import Mesh, PartitionSpec
    from jax.experimental.shard_map import shard_map
    from concourse import bass2jax
    from concourse.bass2jax import _bass_exec_p, partition_id_tensor

    bass2jax.install_neuronx_cc_hook()
    nc = _get_program()
    n_cores = N_RUN_CORES

    partition_name = (nc.partition_id_tensor.name
                      if nc.partition_id_tensor else None)
    in_names, out_names, out_avals, zero_shapes = [], [], [], []
    for alloc in nc.m.functions[0].allocations:
        if not isinstance(alloc, mybir.MemoryLocationSet):
            continue
        name = alloc.memorylocations[0].name
        if alloc.kind == "ExternalInput":
            if name != partition_name:
                in_names.append(name)
        elif alloc.kind == "ExternalOutput":
            out_names.append(name)
            shape = tuple(alloc.tensor_shape)
            dtype = mybir.dt.np(alloc.dtype)
            out_avals.append(jax.core.ShapedArray(shape, dtype))
            zero_shapes.append((shape, dtype))
    n_params = len(in_names)
    n_outs = len(out_avals)
    all_in_names = list(in_names) + list(out_names)
    if partition_name is not None:
        all_in_names.append(partition_name)
    donate = tuple(range(n_params, n_params + n_outs))

    def _body(*args):
        operands = list(args)
        if partition_name is not None:
            operands.append(partition_id_tensor())
        outs = _bass_exec_p.bind(
            *operands, out_avals=tuple(out_avals), in_names=tuple(all_in_names),
            out_names=tuple(out_names), lowering_input_output_aliases=(),
            sim_require_finite=True, sim_require_nnan=True, nc=nc)
        return tuple(outs)

    devices = jax.devices()[:n_cores]
    mesh = Mesh(_np.asarray(devices), ("core",))
    in_specs = (PartitionSpec("core"),) * (n_params + n_outs)
    out_specs = (PartitionSpec("core"),) * len(out_names)
    sharded = jax.jit(
        shard_map(_body, mesh=mesh, in_specs=in_specs, out_specs=out_specs,
                  check_rep=False),
        donate_argnums=donate, keep_unused=True)

    def run(in_maps):
        concat_in = [
            np.concatenate([np.asarray(in_maps[c][nm]) for c in range(n_cores)],
                           axis=0)
            for nm in in_names
        ]
        concat_zeros = [np.zeros((n_cores * s[0], *s[1:]), d)
                        for (s, d) in zero_shapes]
        out_arrs = sharded(*concat_in, *concat_zeros)
        return [
            {nm: np.asarray(out_arrs[i]).reshape(n_cores, *out_avals[i].shape)[c]
             for i, nm in enumerate(out_names)}
            for c in range(n_cores)
        ]

    _CACHE[key] = run
    return run


def _prep_inputs(emb, tw, w1, b1, w2, b2):
    import ml_dtypes
    bfl = ml_dtypes.bfloat16

    w1 = np.asarray(w1, np.float32)
    w1f = np.concatenate([w1[:, 64:].T, 0.5 * w1[:, :64].T], axis=0)  # [128,64]
    w1fd = np.concatenate([w1f, w1f], axis=1).astype(bfl)             # [128,128]

    w2f = np.asarray(w2, np.float32)
    w2d4 = np.zeros((128, 128), np.float32)
    for bb in range(4):
        w2d4[0:64, 32 * bb:32 * bb + 16] = w2f.T
        w2d4[64:128, 32 * bb + 16:32 * bb + 32] = w2f.T
    w2d4 = w2d4.astype(bfl)
    w2t2 = np.zeros((128, 64), np.float32)
    w2t2[0:64, 0:16] = w2f.T          # W2J0: j=0 slot, j=1 zero
    w2t2[64:128, 48:64] = w2f.T       # W2J1: j=1 slot, j=0 zero
    w2t2 = w2t2.astype(bfl)

    b1v = np.asarray(b1, np.float32)
    b2v = np.asarray(b2, np.float32)
    b1s = np.concatenate([b1v, b1v]).reshape(128, 1).astype(np.float32)
    b2s = np.tile(b2v, 8).reshape(128, 1).astype(np.float32)
    return {
        "w1fd": w1fd, "w2d4": w2d4, "w2t2": w2t2, "b1s": b1s, "b2s": b2s,
    }


def _prep_embx(emb, tw):
    """[B, 2, 64, 256] bf16: [b, 0] = (emb*tw).T per batch, [b, 1] = neg."""
    import ml_dtypes
    bfl = ml_dtypes.bfloat16
    x = (emb[:, :-1] * tw[None, :-1]).reshape(-1, L, D)   # [B, 256, 64] f32
    xt = np.ascontiguousarray(x.transpose(0, 2, 1)).astype(bfl)
    return np.ascontiguousarray(
        np.stack([xt, (-xt.astype(np.float32)).astype(bfl)], axis=1))


def kernel(emb, tw, tb, w1, b1, w2, b2, w3, b3, scale):
    run = _get_runner()

    emb = np.asarray(emb, np.float32)
    tw = np.asarray(tw, np.float32)

    shared = _prep_inputs(emb, tw, w1, b1, w2, b2)
    embx = _prep_embx(emb, tw)
    in_maps = []
    for c in range(N_CORES):
        m = dict(shared)
        m["embx"] = np.ascontiguousarray(
            embx[c * B_PER_CORE:(c + 1) * B_PER_CORE])
        in_maps.append(m)

    core_results = run(in_maps[:N_RUN_CORES])

    x1 = emb @ tw + float(tb[0])  # [32] f32 on host
    w3v = np.asarray(w3, np.float32)[0]
    b2v = np.asarray(b2, np.float32)
    # zero-filled block contributes 128*lrelu(b2) to lanes 112:128 col 7
    zero_corr = 128.0 * np.where(b2v > 0, b2v, 0.01 * b2v)
    out = np.zeros(32, np.float32)
    for c in range(N_RUN_CORES):
        acc = core_results[c]["acc_o"]   # [4, 128, 8]
        for b in range(N_BATCH):
            R = acc[b].reshape(8, 16, 8).sum(axis=(0, 2)) - zero_corr
            out[c * B_PER_CORE + b] = (
                x1[c * B_PER_CORE + b]
                + float(scale[0]) * (R @ w3v + float(b3[0]) * NPAIRS)
            )
    return out


# revision 26
# speedup vs baseline: 1.1329x; 1.1329x over previous
"""Trainium2 Bass kernel for nn_EpiNN_aaindex (pairwise-MLP GNN reduction).

Math (per batch b):
  x1 = emb@tw + tb                              (computed on HOST, f32)
  X[i,d] = emb[i*64+d] * tw[i*64+d]             (L=256, D=64; on HOST, bf16)
  s_ij = MLP(concat[(x_i+x_j)/2, |x_i-x_j|])    (128->64->16->1, LeakyReLU)
  out_b = x1 + scale * sum_{i<j} s_ij

Strategy: 8 cores, 4 batches/core (data parallel over B=32).
Exact upper-triangle enumeration via cyclic offsets o=1..128:
pairs (i, (i+o) mod 256) for o=1..127 cover each unordered pair once;
o=128's second half (i>=128) is excluded on-device via a split L2 matmul.

Device layouts (per batch, SBUF, bf16; X2T = X.T [64 d, 256 i], from host):
  XU  [128, 256]: both lane halves = X2T
  XSN [128, 512]: lanes 0:64 = X2T|X2T, lanes 64:128 = -(X2T|X2T)
Main loop: flat list of 32 (batch, p) pairs, 16 offsets each:
o = 8p+1+b (+64j), j in {0,1}, b in 0..7 (half A: b 0..3, B: b 4..7).
  M [128, 16, 256] = XU - XSN[:, win] (ONE DVE op; overlapping windows via
    custom AP [[64,2],[1,8],[1,256]] starting at col 8p+1):
    block m=8j+b: lanes 0:64 = x_i - x_{i+o}, lanes 64:128 = x_i + x_{i+o}
  abs on lanes 0:64 (u16 bitmask), then per half 4 512-free 128-contraction
  matmuls, stationary [w1b.T ; 0.5*w1a.T] -> P1 [128, 1024]
  (lanes 0:64 = j=0 offsets, 64:128 = j=1);
  act1 Lrelu+b1 -> H1 bf16 [128, 1024];
  L2: 4 matmuls, stationary = block-diag w2.T pairs -> P2D [128, 512]
  (half A cols 0:256, B 256:512; lane r: b=r//32, j=(r%32)//16, f=r%16);
  act2 Lrelu+b2 -> HJ, then DVE tensor_reduce -> ACC[:, p].
L2 and act2 are emitted one pair behind L1/act1 (software pipelining) so
the PE and Act engines never stall on same-pair dependencies.
o=128 (p=7, B, b=7, j=1): its L2 only covers i<128; cols 384:512 of lanes
112:128 are zero-filled, adding exactly 128*lrelu(b2) to ACC[112:128, 7]
(host subtracts).

Final combine on host: out = x1 + scale*(w3 . R + 32640*b3).
"""
import numpy as np

L, D = 256, 64
B_PER_CORE = 4
N_CORES = 8
NPAIRS = 32640  # 256*255/2

_CACHE = {}
import os as _os
N_BATCH = int(_os.environ.get("EPINN_BATCH", str(B_PER_CORE)))
N_RUN_CORES = int(_os.environ.get("EPINN_CORES", str(N_CORES)))


def _build_program():
    import concourse.bacc as bacc
    import concourse.mybir as mybir
    import concourse.tile as tile
    from concourse.bass import AP
    from contextlib import ExitStack

    f32 = mybir.dt.float32
    bf16 = mybir.dt.bfloat16
    u16 = mybir.dt.uint16
    AF = mybir.ActivationFunctionType
    ALU = mybir.AluOpType

    nc = bacc.Bacc("TRN2", target_bir_lowering=False, debug=False,
                   num_devices=N_CORES)

    # ---- DRAM parameters (per core) ----
    # embx[b] = [2, 64, 256]: [0] = X2T (bf16), [1] = -X2T
    embx_d = nc.declare_dram_parameter("embx", [B_PER_CORE, 2, D, L], bf16,
                                       isOutput=False)
    w1fd_d = nc.declare_dram_parameter("w1fd", [128, 128], bf16, isOutput=False)
    w2d4_d = nc.declare_dram_parameter("w2d4", [128, 128], bf16, isOutput=False)
    w2t_d = nc.declare_dram_parameter("w2t2", [128, 64], bf16, isOutput=False)
    b1s_d = nc.declare_dram_parameter("b1s", [128, 1], f32, isOutput=False)
    b2s_d = nc.declare_dram_parameter("b2s", [128, 1], f32, isOutput=False)

    acc_o = nc.declare_dram_parameter("acc_o", [B_PER_CORE, 128, 8], f32,
                                      isOutput=True)

    with tile.TileContext(nc) as tc, ExitStack() as ctx:
        cpool = ctx.enter_context(tc.tile_pool(name="consts", bufs=1))
        ppool = ctx.enter_context(tc.tile_pool(name="persist", bufs=1))
        mpool = ctx.enter_context(tc.tile_pool(name="mbufs", bufs=3))
        hpool = ctx.enter_context(tc.tile_pool(name="hbufs", bufs=4))
        jpool = ctx.enter_context(tc.tile_pool(name="junk", bufs=2))
        pp1 = ctx.enter_context(tc.tile_pool(name="p1", bufs=3, space="PSUM"))
        pp2 = ctx.enter_context(tc.tile_pool(name="p2", bufs=2, space="PSUM"))

        # ---- static weights / consts ----
        W1FD = cpool.tile([128, 128], bf16)
        W2D4 = cpool.tile([128, 128], bf16)
        W2T = cpool.tile([128, 64], bf16)
        B1S = cpool.tile([128, 1], f32)
        B2S = cpool.tile([128, 1], f32)

        nc.sync.dma_start(W1FD[:], w1fd_d[:])
        nc.sync.dma_start(W2D4[:], w2d4_d[:])
        nc.sync.dma_start(W2T[:], w2t_d[:])
        nc.sync.dma_start(B1S[:], b1s_d[:])
        nc.sync.dma_start(B2S[:], b2s_d[:])

        XUs = [None] * N_BATCH
        XSNs = [None] * N_BATCH
        ACCs = [None] * N_BATCH

        def emit_setup(b):
            XU = ppool.tile([128, 256], bf16, name=f"xu{b}")
            XSN = ppool.tile([128, 512], bf16, name=f"xsn{b}")
            src = embx_d[b].rearrange("g d f -> (g d) f")  # [128, 256]
            nc.sync.dma_start(XSN[:, 0:256], src)
            nc.sync.dma_start(XSN[:, 256:512], src)
            nc.sync.dma_start(XU[0:64, :], embx_d[b, 0])
            nc.sync.dma_start(XU[64:128, :], embx_d[b, 0])
            ACC = ppool.tile([128, 8], f32, name=f"acc{b}")
            nc.gpsimd.memset(ACC[:], 0.0)
            XUs[b] = XU
            XSNs[b] = XSN
            ACCs[b] = ACC

        def emit_front(b, p):
            """sub/abs + L1 + act1 for pair (b, p); returns H1 tiles."""
            XU, XSN = XUs[b], XSNs[b]
            pstride = XU[:].ap.copy()[0][0]
            XU_B = AP(XU.tensor, XU.offset,
                      [[pstride, 128], [0, 2], [0, 8], [1, 256]])
            xsn_pstride = XSN[:].ap.copy()[0][0]

            o0 = 8 * p + 1
            M = mpool.tile([128, 16, 256], bf16, tag="m")
            MW = M[:].rearrange("p (a b) f -> p a b f", a=2)
            XSN_W = AP(XSN.tensor, XSN.offset + o0,
                       [[xsn_pstride, 128], [64, 2], [1, 8], [1, 256]])
            nc.vector.tensor_tensor(out=MW, in0=XU_B, in1=XSN_W,
                                    op=ALU.subtract)
            nc.vector.tensor_scalar(
                out=M[0:64, :, :].bitcast(u16),
                in0=M[0:64, :, :].bitcast(u16),
                scalar1=0x7FFF, scalar2=None, op0=ALU.bitwise_and)

            H1s = []
            for h in (0, 1):  # half A/B
                P1 = pp1.tile([128, 1024], f32, tag="p1")
                for j in (0, 1):
                    lhs = W1FD[:, 64 * j:64 * j + 64]
                    for bb in (0, 2):
                        m0 = 8 * j + 4 * h + bb
                        nc.tensor.matmul(
                            P1[64 * j:64 * j + 64, 256 * bb:256 * bb + 512],
                            lhs, M[:, m0:m0 + 2, :],
                            start=True, stop=True, skip_group_check=True)
                H1 = hpool.tile([128, 1024], bf16, tag="h1")
                nc.scalar.activation(H1[:], P1[:], AF.Lrelu, bias=B1S[:],
                                     scale=1.0, alpha=0.01)
                H1s.append(H1)
            return H1s

        def emit_back(b, p, H1s):
            """L2 + act2 + accum for pair (b, p)."""
            ACC = ACCs[b]
            P2D = pp2.tile([128, 512], f32, tag="p2")
            for h in (0, 1):
                H1 = H1s[h]
                for bb in range(4):
                    if p == 7 and h == 1 and bb == 3:
                        nc.tensor.matmul(
                            P2D[96:128, 256:512], W2T[:, 0:32],
                            H1[:, 768:1024],
                            start=True, stop=False, skip_group_check=True,
                            tile_position=(0, 96))
                        nc.tensor.matmul(
                            P2D[96:128, 256:384], W2T[:, 32:64],
                            H1[:, 768:896],
                            start=False, stop=True, skip_group_check=True,
                            tile_position=(0, 96))
                    else:
                        nc.tensor.matmul(
                            P2D[32 * bb:32 * bb + 32, 256 * h:256 * h + 256],
                            W2D4[:, 32 * bb:32 * bb + 32],
                            H1[:, 256 * bb:256 * bb + 256],
                            start=True, stop=True, skip_group_check=True,
                            tile_position=(0, 32 * bb))

            HJ = jpool.tile([128, 512], bf16, tag="hj")
            nc.scalar.activation(HJ[:], P2D[:], AF.Lrelu, bias=B2S[:],
                                 scale=1.0, alpha=0.01,
                                 accum_out=ACC[:, p:p + 1])

        # ---- software-pipelined emission over the flat pair list ----
        emit_setup(0)
        pairs = [(b, p) for b in range(N_BATCH) for p in range(8)]
        prev = None
        for k, (b, p) in enumerate(pairs):
            H1s = emit_front(b, p)
            if prev is not None:
                pb, pp, pH = prev
                emit_back(pb, pp, pH)
                if pp == 7:
                    nc.sync.dma_start(acc_o[pb], ACCs[pb][:])
            prev = (b, p, H1s)
            if p == 0 and b + 1 < N_BATCH:
                emit_setup(b + 1)
        pb, pp, pH = prev
        emit_back(pb, pp, pH)
        nc.sync.dma_start(acc_o[pb], ACCs[pb][:])

    nc.compile()
    return nc


def _get_program():
    key = ("prog", N_BATCH)
    if key not in _CACHE:
        _CACHE[key] = _build_program()
    return _CACHE[key]


def _get_runner():
    """Build (once) a cached jitted SPMD executable for the program."""
    key = ("runner", N_BATCH, N_RUN_CORES)
    if key in _CACHE:
        return _CACHE[key]
    import jax
    import numpy as _np
    import concourse.mybir as mybir
    from jax.sharding import Mesh, PartitionSpec
    from jax.experimental.shard_map import shard_map
    from concourse import bass2jax
    from concourse.bass2jax import _bass_exec_p, partition_id_tensor

    bass2jax.install_neuronx_cc_hook()
    nc = _get_program()
    n_cores = N_RUN_CORES

    partition_name = (nc.partition_id_tensor.name
                      if nc.partition_id_tensor else None)
    in_names, out_names, out_avals, zero_shapes = [], [], [], []
    for alloc in nc.m.functions[0].allocations:
        if not isinstance(alloc, mybir.MemoryLocationSet):
            continue
        name = alloc.memorylocations[0].name
        if alloc.kind == "ExternalInput":
            if name != partition_name:
                in_names.append(name)
        elif alloc.kind == "ExternalOutput":
            out_names.append(name)
            shape = tuple(alloc.tensor_shape)
            dtype = mybir.dt.np(alloc.dtype)
            out_avals.append(jax.core.ShapedArray(shape, dtype))
            zero_shapes.append((shape, dtype))
    n_params = len(in_names)
    n_outs = len(out_avals)
    all_in_names = list(in_names) + list(out_names)
    if partition_name is not None:
        all_in_names.append(partition_name)
    donate = tuple(range(n_params, n_params + n_outs))

    def _body(*args):
        operands = list(args)
        if partition_name is not None:
            operands.append(partition_id_tensor())
        outs = _bass_exec_p.bind(
            *operands, out_avals=tuple(out_avals), in_names=tuple(all_in_names),
            out_names=tuple(out_names), lowering_input_output_aliases=(),
            sim_require_finite=True, sim_require_nnan=True, nc=nc)
        return tuple(outs)

    devices = jax.devices()[:n_cores]
    mesh = Mesh(_np.asarray(devices), ("core",))
    in_specs = (PartitionSpec("core"),) * (n_params + n_outs)
    out_specs = (PartitionSpec("core"),) * len(out_names)
    sharded = jax.jit(
        shard_map(_body, mesh=mesh, in_specs=in_specs, out_specs=out_specs,
                  check_rep=False),
        donate_argnums=donate, keep_unused=True)

    def run(in_maps):
        concat_in = [
            np.concatenate([np.asarray(in_maps[c][nm]) for c in range(n_cores)],
                           axis=0)
            for nm in in_names
        ]
        concat_zeros = [np.zeros((n_cores * s[0], *s[1:]), d)
                        for (s, d) in zero_shapes]
        out_arrs = sharded(*concat_in, *concat_zeros)
        return [
            {nm: np.asarray(out_arrs[i]).reshape(n_cores, *out_avals[i].shape)[c]
             for i, nm in enumerate(out_names)}
            for c in range(n_cores)
        ]

    _CACHE[key] = run
    return run


def _prep_inputs(emb, tw, w1, b1, w2, b2):
    import ml_dtypes
    bfl = ml_dtypes.bfloat16

    w1 = np.asarray(w1, np.float32)
    w1f = np.concatenate([w1[:, 64:].T, 0.5 * w1[:, :64].T], axis=0)  # [128,64]
    w1fd = np.concatenate([w1f, w1f], axis=1).astype(bfl)             # [128,128]

    w2f = np.asarray(w2, np.float32)
    w2d4 = np.zeros((128, 128), np.float32)
    for bb in range(4):
        w2d4[0:64, 32 * bb:32 * bb + 16] = w2f.T
        w2d4[64:128, 32 * bb + 16:32 * bb + 32] = w2f.T
    w2d4 = w2d4.astype(bfl)
    w2t2 = np.zeros((128, 64), np.float32)
    w2t2[0:64, 0:16] = w2f.T          # W2J0: j=0 slot, j=1 zero
    w2t2[64:128, 48:64] = w2f.T       # W2J1: j=1 slot, j=0 zero
    w2t2 = w2t2.astype(bfl)

    b1v = np.asarray(b1, np.float32)
    b2v = np.asarray(b2, np.float32)
    b1s = np.concatenate([b1v, b1v]).reshape(128, 1).astype(np.float32)
    b2s = np.tile(b2v, 8).reshape(128, 1).astype(np.float32)
    return {
        "w1fd": w1fd, "w2d4": w2d4, "w2t2": w2t2, "b1s": b1s, "b2s": b2s,
    }


def _prep_embx(emb, tw):
    """[B, 2, 64, 256] bf16: [b, 0] = (emb*tw).T per batch, [b, 1] = neg."""
    import ml_dtypes
    bfl = ml_dtypes.bfloat16
    x = (emb[:, :-1] * tw[None, :-1]).reshape(-1, L, D)   # [B, 256, 64] f32
    xt = np.ascontiguousarray(x.transpose(0, 2, 1)).astype(bfl)
    return np.ascontiguousarray(
        np.stack([xt, (-xt.astype(np.float32)).astype(bfl)], axis=1))


def kernel(emb, tw, tb, w1, b1, w2, b2, w3, b3, scale):
    run = _get_runner()

    emb = np.asarray(emb, np.float32)
    tw = np.asarray(tw, np.float32)

    shared = _prep_inputs(emb, tw, w1, b1, w2, b2)
    embx = _prep_embx(emb, tw)
    in_maps = []
    for c in range(N_CORES):
        m = dict(shared)
        m["embx"] = np.ascontiguousarray(
            embx[c * B_PER_CORE:(c + 1) * B_PER_CORE])
        in_maps.append(m)

    core_results = run(in_maps[:N_RUN_CORES])

    x1 = emb @ tw + float(tb[0])  # [32] f32 on host
    w3v = np.asarray(w3, np.float32)[0]
    b2v = np.asarray(b2, np.float32)
    # zero-filled block contributes 128*lrelu(b2) to lanes 112:128 col 7
    zero_corr = 128.0 * np.where(b2v > 0, b2v, 0.01 * b2v)
    out = np.zeros(32, np.float32)
    for c in range(N_RUN_CORES):
        acc = core_results[c]["acc_o"]   # [4, 128, 8]
        for b in range(N_BATCH):
            R = acc[b].reshape(8, 16, 8).sum(axis=(0, 2)) - zero_corr
            out[c * B_PER_CORE + b] = (
                x1[c * B_PER_CORE + b]
                + float(scale[0]) * (R @ w3v + float(b3[0]) * NPAIRS)
            )
    return out


# revision 32
# speedup vs baseline: 1.2579x; 1.1103x over previous
"""Trainium2 Bass kernel for nn_EpiNN_aaindex (pairwise-MLP GNN reduction).

Math (per batch b):
  x1 = emb@tw + tb                              (computed on HOST, f32)
  X[i,d] = emb[i*64+d] * tw[i*64+d]             (L=256, D=64; on HOST, bf16)
  s_ij = MLP(concat[(x_i+x_j)/2, |x_i-x_j|])    (128->64->16->1, LeakyReLU)
  out_b = x1 + scale * sum_{i<j} s_ij

Strategy: 8 cores, 4 batches/core (data parallel over B=32).
Exact upper-triangle enumeration via cyclic offsets o=1..128:
pairs (i, (i+o) mod 256) for o=1..127 cover each unordered pair once;
o=128's second half (i>=128) is excluded on-device via a split L2 matmul.

Device layouts (per batch, SBUF, bf16; X2T = X.T [64 d, 256 i], from host):
  XU  [128, 256]: both lane halves = X2T
  XSN [128, 512]: lanes 0:64 = X2T|X2T, lanes 64:128 = -(X2T|X2T)
Main loop: flat list of 32 (batch, p) pairs, 16 offsets each:
o = 8p+1+b (+64j), j in {0,1}, b in 0..7 (half A: b 0..3, B: b 4..7).
  M [128, 16, 256] = XU - XSN[:, win] (ONE DVE op; overlapping windows via
    custom AP [[64,2],[1,8],[1,256]] starting at col 8p+1):
    block m=8j+b: lanes 0:64 = x_i - x_{i+o}, lanes 64:128 = x_i + x_{i+o}
  abs on lanes 0:64 (u16 bitmask), then per half 4 512-free 128-contraction
  matmuls, stationary [w1b.T ; 0.5*w1a.T] -> P1 [128, 1024]
  (lanes 0:64 = j=0 offsets, 64:128 = j=1);
  act1 Lrelu+b1 -> H1 bf16 [128, 1024];
  L2: 4 matmuls, stationary = block-diag w2.T pairs -> P2D [128, 512]
  (half A cols 0:256, B 256:512; lane r: b=r//32, j=(r%32)//16, f=r%16);
  act2 Lrelu+b2 -> HJ, then DVE tensor_reduce -> ACC[:, p].
L2 and act2 are emitted one pair behind L1/act1 (software pipelining) so
the PE and Act engines never stall on same-pair dependencies.
o=128 (p=7, B, b=7, j=1): its L2 only covers i<128; cols 384:512 of lanes
112:128 are zero-filled, adding exactly 128*lrelu(b2) to ACC[112:128, 7]
(host subtracts).

Final combine on host: out = x1 + scale*(w3 . R + 32640*b3).
"""
import numpy as np

L, D = 256, 64
B_PER_CORE = 4
N_CORES = 8
NPAIRS = 32640  # 256*255/2

_CACHE = {}
import os as _os
N_BATCH = int(_os.environ.get("EPINN_BATCH", str(B_PER_CORE)))
N_RUN_CORES = int(_os.environ.get("EPINN_CORES", str(N_CORES)))


def _build_program():
    import concourse.bacc as bacc
    import concourse.mybir as mybir
    import concourse.tile as tile
    from concourse.bass import AP
    from contextlib import ExitStack

    f32 = mybir.dt.float32
    bf16 = mybir.dt.bfloat16
    u16 = mybir.dt.uint16
    AF = mybir.ActivationFunctionType
    ALU = mybir.AluOpType

    nc = bacc.Bacc("TRN2", target_bir_lowering=False, debug=False,
                   num_devices=N_CORES)

    # ---- DRAM parameters (per core) ----
    # embx[b] = [2, 64, 256]: [0] = X2T (bf16), [1] = X2T rolled by -64
    embx_d = nc.declare_dram_parameter("embx", [B_PER_CORE, 2, D, L], bf16,
                                       isOutput=False)
    w1sa_d = nc.declare_dram_parameter("w1sa", [128, 128], bf16, isOutput=False)
    w1sb_d = nc.declare_dram_parameter("w1sb", [128, 128], bf16, isOutput=False)
    w2d4_d = nc.declare_dram_parameter("w2d4", [128, 128], bf16, isOutput=False)
    w2t_d = nc.declare_dram_parameter("w2t2", [128, 64], bf16, isOutput=False)
    b1s_d = nc.declare_dram_parameter("b1s", [128, 1], f32, isOutput=False)
    b2s_d = nc.declare_dram_parameter("b2s", [128, 1], f32, isOutput=False)

    acc_o = nc.declare_dram_parameter("acc_o", [B_PER_CORE, 128, 8], f32,
                                      isOutput=True)

    with tile.TileContext(nc) as tc, ExitStack() as ctx:
        cpool = ctx.enter_context(tc.tile_pool(name="consts", bufs=1))
        ppool = ctx.enter_context(tc.tile_pool(name="persist", bufs=1))
        mpool = ctx.enter_context(tc.tile_pool(name="mbufs", bufs=3))
        hpool = ctx.enter_context(tc.tile_pool(name="hbufs", bufs=4))
        jpool = ctx.enter_context(tc.tile_pool(name="junk", bufs=2))
        pp1 = ctx.enter_context(tc.tile_pool(name="p1", bufs=3, space="PSUM"))
        pp2 = ctx.enter_context(tc.tile_pool(name="p2", bufs=2, space="PSUM"))

        # ---- static weights / consts ----
        W1SA = cpool.tile([128, 128], bf16)
        W1SB = cpool.tile([128, 128], bf16)
        W2D4 = cpool.tile([128, 128], bf16)
        W2T = cpool.tile([128, 64], bf16)
        B1S = cpool.tile([128, 1], f32)
        B2S = cpool.tile([128, 1], f32)

        nc.sync.dma_start(W1SA[:], w1sa_d[:])
        nc.sync.dma_start(W1SB[:], w1sb_d[:])
        nc.sync.dma_start(W2D4[:], w2d4_d[:])
        nc.sync.dma_start(W2T[:], w2t_d[:])
        nc.sync.dma_start(B1S[:], b1s_d[:])
        nc.sync.dma_start(B2S[:], b2s_d[:])

        XUs = [None] * N_BATCH
        XSNs = [None] * N_BATCH
        ACCs = [None] * N_BATCH

        def emit_setup(b):
            XU = ppool.tile([128, 256], bf16, name=f"xu{b}")
            XSP = ppool.tile([128, 512], bf16, name=f"xsp{b}")
            nc.sync.dma_start(XSP[0:64, 0:256], embx_d[b, 0])
            nc.sync.dma_start(XSP[0:64, 256:512], embx_d[b, 0])
            nc.sync.dma_start(XSP[64:128, 0:256], embx_d[b, 1])
            nc.sync.dma_start(XSP[64:128, 256:512], embx_d[b, 1])
            nc.sync.dma_start(XU[0:64, :], embx_d[b, 0])
            nc.sync.dma_start(XU[64:128, :], embx_d[b, 0])
            ACC = ppool.tile([128, 8], f32, name=f"acc{b}")
            nc.gpsimd.memset(ACC[:], 0.0)
            XUs[b] = XU
            XSNs[b] = XSP
            ACCs[b] = ACC

        def emit_front(b, p):
            """sub/add/abs + L1 + act1 for pair (b, p); returns H1 tiles."""
            XU, XSP = XUs[b], XSNs[b]
            pstride = XU[:].ap.copy()[0][0]
            XU_B = AP(XU.tensor, XU.offset,
                      [[pstride, 128], [0, 8], [1, 256]])
            xsp_pstride = XSP[:].ap.copy()[0][0]

            o0 = 8 * p + 1
            # D2 block c: lanes 0:64 = x_i - x_{i+o0+c}, 64:128 = (o0+c+64)
            # S2 block c: same offsets, x_i + x_j
            D2 = mpool.tile([128, 8, 256], bf16, tag="d2")
            S2 = mpool.tile([128, 8, 256], bf16, tag="s2")
            XSP_W = AP(XSP.tensor, XSP.offset + o0,
                       [[xsp_pstride, 128], [1, 8], [1, 256]])
            nc.vector.tensor_tensor(out=D2[:], in0=XU_B, in1=XSP_W,
                                    op=ALU.subtract)
            nc.vector.tensor_tensor(out=S2[:], in0=XU_B, in1=XSP_W,
                                    op=ALU.add)
            nc.vector.tensor_scalar(
                out=D2[:].bitcast(u16), in0=D2[:].bitcast(u16),
                scalar1=0x7FFF, scalar2=None, op0=ALU.bitwise_and)

            H1s = []
            for h in (0, 1):  # half A/B
                P1 = pp1.tile([128, 1024], f32, tag="p1")
                for c2 in (0, 2):
                    nc.tensor.matmul(
                        P1[:, 256 * c2:256 * c2 + 512], W1SA[:],
                        D2[:, 4 * h + c2:4 * h + c2 + 2, :],
                        start=True, stop=False, skip_group_check=True)
                for c2 in (0, 2):
                    nc.tensor.matmul(
                        P1[:, 256 * c2:256 * c2 + 512], W1SB[:],
                        S2[:, 4 * h + c2:4 * h + c2 + 2, :],
                        start=False, stop=True, skip_group_check=True)
                H1 = hpool.tile([128, 1024], bf16, tag="h1")
                nc.scalar.activation(H1[:], P1[:], AF.Lrelu, bias=B1S[:],
                                     scale=1.0, alpha=0.01)
                H1s.append(H1)
            return H1s

        def emit_back(b, p, H1s):
            """L2 + act2 + accum for pair (b, p)."""
            ACC = ACCs[b]
            P2D = pp2.tile([128, 512], f32, tag="p2")
            for h in (0, 1):
                H1 = H1s[h]
                for bb in range(4):
                    if p == 7 and h == 1 and bb == 3:
                        nc.tensor.matmul(
                            P2D[96:128, 256:512], W2T[:, 0:32],
                            H1[:, 768:1024],
                            start=True, stop=False, skip_group_check=True,
                            tile_position=(0, 96))
                        nc.tensor.matmul(
                            P2D[96:128, 256:384], W2T[:, 32:64],
                            H1[:, 768:896],
                            start=False, stop=True, skip_group_check=True,
                            tile_position=(0, 96))
                    else:
                        nc.tensor.matmul(
                            P2D[32 * bb:32 * bb + 32, 256 * h:256 * h + 256],
                            W2D4[:, 32 * bb:32 * bb + 32],
                            H1[:, 256 * bb:256 * bb + 256],
                            start=True, stop=True, skip_group_check=True,
                            tile_position=(0, 32 * bb))

            HJ = jpool.tile([128, 512], bf16, tag="hj")
            nc.scalar.activation(HJ[:], P2D[:], AF.Lrelu, bias=B2S[:],
                                 scale=1.0, alpha=0.01,
                                 accum_out=ACC[:, p:p + 1])

        # ---- software-pipelined emission over the flat pair list ----
        emit_setup(0)
        pairs = [(b, p) for b in range(N_BATCH) for p in range(8)]
        prev = None
        for k, (b, p) in enumerate(pairs):
            H1s = emit_front(b, p)
            if prev is not None:
                pb, pp, pH = prev
                emit_back(pb, pp, pH)
                if pp == 7:
                    nc.sync.dma_start(acc_o[pb], ACCs[pb][:])
            prev = (b, p, H1s)
            if p == 0 and b + 1 < N_BATCH:
                emit_setup(b + 1)
        pb, pp, pH = prev
        emit_back(pb, pp, pH)
        nc.sync.dma_start(acc_o[pb], ACCs[pb][:])

    nc.compile()
    return nc


def _get_program():
    key = ("prog", N_BATCH)
    if key not in _CACHE:
        _CACHE[key] = _build_program()
    return _CACHE[key]


def _get_runner():
    """Build (once) a cached jitted SPMD executable for the program."""
    key = ("runner", N_BATCH, N_RUN_CORES)
    if key in _CACHE:
        return _CACHE[key]
    import jax
    import numpy as _np
    import concourse.mybir as mybir
    from jax.sharding import Mesh, PartitionSpec
    from jax.experimental.shard_map import shard_map
    from concourse import bass2jax
    from concourse.bass2jax import _bass_exec_p, partition_id_tensor

    bass2jax.install_neuronx_cc_hook()
    nc = _get_program()
    n_cores = N_RUN_CORES

    partition_name = (nc.partition_id_tensor.name
                      if nc.partition_id_tensor else None)
    in_names, out_names, out_avals, zero_shapes = [], [], [], []
    for alloc in nc.m.functions[0].allocations:
        if not isinstance(alloc, mybir.MemoryLocationSet):
            continue
        name = alloc.memorylocations[0].name
        if alloc.kind == "ExternalInput":
            if name != partition_name:
                in_names.append(name)
        elif alloc.kind == "ExternalOutput":
            out_names.append(name)
            shape = tuple(alloc.tensor_shape)
            dtype = mybir.dt.np(alloc.dtype)
            out_avals.append(jax.core.ShapedArray(shape, dtype))
            zero_shapes.append((shape, dtype))
    n_params = len(in_names)
    n_outs = len(out_avals)
    all_in_names = list(in_names) + list(out_names)
    if partition_name is not None:
        all_in_names.append(partition_name)
    donate = tuple(range(n_params, n_params + n_outs))

    def _body(*args):
        operands = list(args)
        if partition_name is not None:
            operands.append(partition_id_tensor())
        outs = _bass_exec_p.bind(
            *operands, out_avals=tuple(out_avals), in_names=tuple(all_in_names),
            out_names=tuple(out_names), lowering_input_output_aliases=(),
            sim_require_finite=True, sim_require_nnan=True, nc=nc)
        return tuple(outs)

    devices = jax.devices()[:n_cores]
    mesh = Mesh(_np.asarray(devices), ("core",))
    in_specs = (PartitionSpec("core"),) * (n_params + n_outs)
    out_specs = (PartitionSpec("core"),) * len(out_names)
    sharded = jax.jit(
        shard_map(_body, mesh=mesh, in_specs=in_specs, out_specs=out_specs,
                  check_rep=False),
        donate_argnums=donate, keep_unused=True)

    def run(in_maps):
        concat_in = [
            np.concatenate([np.asarray(in_maps[c][nm]) for c in range(n_cores)],
                           axis=0)
            for nm in in_names
        ]
        concat_zeros = [np.zeros((n_cores * s[0], *s[1:]), d)
                        for (s, d) in zero_shapes]
        out_arrs = sharded(*concat_in, *concat_zeros)
        return [
            {nm: np.asarray(out_arrs[i]).reshape(n_cores, *out_avals[i].shape)[c]
             for i, nm in enumerate(out_names)}
            for c in range(n_cores)
        ]

    _CACHE[key] = run
    return run


def _prep_inputs(emb, tw, w1, b1, w2, b2):
    import ml_dtypes
    bfl = ml_dtypes.bfloat16

    w1 = np.asarray(w1, np.float32)
    w1bt = w1[:, 64:].T               # [64, 64] abs-diff part
    w1at = 0.5 * w1[:, :64].T         # [64, 64] sum part
    w1sa = np.zeros((128, 128), np.float32)
    w1sa[0:64, 0:64] = w1bt
    w1sa[64:128, 64:128] = w1bt
    w1sb = np.zeros((128, 128), np.float32)
    w1sb[0:64, 0:64] = w1at
    w1sb[64:128, 64:128] = w1at
    w1sa = w1sa.astype(bfl)
    w1sb = w1sb.astype(bfl)

    w2f = np.asarray(w2, np.float32)
    w2d4 = np.zeros((128, 128), np.float32)
    for bb in range(4):
        w2d4[0:64, 32 * bb:32 * bb + 16] = w2f.T
        w2d4[64:128, 32 * bb + 16:32 * bb + 32] = w2f.T
    w2d4 = w2d4.astype(bfl)
    w2t2 = np.zeros((128, 64), np.float32)
    w2t2[0:64, 0:16] = w2f.T          # W2J0: j=0 slot, j=1 zero
    w2t2[64:128, 48:64] = w2f.T       # W2J1: j=1 slot, j=0 zero
    w2t2 = w2t2.astype(bfl)

    b1v = np.asarray(b1, np.float32)
    b2v = np.asarray(b2, np.float32)
    b1s = np.concatenate([b1v, b1v]).reshape(128, 1).astype(np.float32)
    b2s = np.tile(b2v, 8).reshape(128, 1).astype(np.float32)
    return {
        "w1sa": w1sa, "w1sb": w1sb, "w2d4": w2d4, "w2t2": w2t2,
        "b1s": b1s, "b2s": b2s,
    }


def _prep_embx(emb, tw):
    """[B, 2, 64, 256] bf16: [b,0] = (emb*tw).T, [b,1] = same rolled by -64."""
    import ml_dtypes
    bfl = ml_dtypes.bfloat16
    x = (emb[:, :-1] * tw[None, :-1]).reshape(-1, L, D)   # [B, 256, 64] f32
    xt = np.ascontiguousarray(x.transpose(0, 2, 1)).astype(bfl)
    xr = np.concatenate([xt[..., 64:], xt[..., :64]], axis=-1)
    return np.ascontiguousarray(np.stack([xt, xr], axis=1))


def kernel(emb, tw, tb, w1, b1, w2, b2, w3, b3, scale):
    run = _get_runner()

    emb = np.asarray(emb, np.float32)
    tw = np.asarray(tw, np.float32)

    shared = _prep_inputs(emb, tw, w1, b1, w2, b2)
    embx = _prep_embx(emb, tw)
    in_maps = []
    for c in range(N_CORES):
        m = dict(shared)
        m["embx"] = np.ascontiguousarray(
            embx[c * B_PER_CORE:(c + 1) * B_PER_CORE])
        in_maps.append(m)

    core_results = run(in_maps[:N_RUN_CORES])

    x1 = emb @ tw + float(tb[0])  # [32] f32 on host
    w3v = np.asarray(w3, np.float32)[0]
    b2v = np.asarray(b2, np.float32)
    # zero-filled block contributes 128*lrelu(b2) to lanes 112:128 col 7
    zero_corr = 128.0 * np.where(b2v > 0, b2v, 0.01 * b2v)
    out = np.zeros(32, np.float32)
    for c in range(N_RUN_CORES):
        acc = core_results[c]["acc_o"]   # [4, 128, 8]
        for b in range(N_BATCH):
            R = acc[b].reshape(8, 16, 8).sum(axis=(0, 2)) - zero_corr
            out[c * B_PER_CORE + b] = (
                x1[c * B_PER_CORE + b]
                + float(scale[0]) * (R @ w3v + float(b3[0]) * NPAIRS)
            )
    return out


# revision 36
# speedup vs baseline: 1.3173x; 1.0472x over previous
"""Trainium2 Bass kernel for nn_EpiNN_aaindex (pairwise-MLP GNN reduction).

Math (per batch b):
  x1 = emb@tw + tb                              (computed on HOST, f32)
  X[i,d] = emb[i*64+d] * tw[i*64+d]             (L=256, D=64; on HOST, bf16)
  s_ij = MLP(concat[(x_i+x_j)/2, |x_i-x_j|])    (128->64->16->1, LeakyReLU)
  out_b = x1 + scale * sum_{i<j} s_ij

Strategy: 8 cores, 4 batches/core (data parallel over B=32).
Exact upper-triangle enumeration via cyclic offsets o=1..128:
pairs (i, (i+o) mod 256) for o=1..127 cover each unordered pair once;
o=128's second half (i>=128) is excluded on-device via a split L2 matmul.

Device layouts (per batch, SBUF, bf16; X2T = X.T [64 d, 256 i], from host):
  XU  [128, 256]: both lane halves = X2T
  XSN [128, 512]: lanes 0:64 = X2T|X2T, lanes 64:128 = -(X2T|X2T)
Main loop: flat list of 32 (batch, p) pairs, 16 offsets each:
o = 8p+1+b (+64j), j in {0,1}, b in 0..7 (half A: b 0..3, B: b 4..7).
  M [128, 16, 256] = XU - XSN[:, win] (ONE DVE op; overlapping windows via
    custom AP [[64,2],[1,8],[1,256]] starting at col 8p+1):
    block m=8j+b: lanes 0:64 = x_i - x_{i+o}, lanes 64:128 = x_i + x_{i+o}
  abs on lanes 0:64 (u16 bitmask), then per half 4 512-free 128-contraction
  matmuls, stationary [w1b.T ; 0.5*w1a.T] -> P1 [128, 1024]
  (lanes 0:64 = j=0 offsets, 64:128 = j=1);
  act1 Lrelu+b1 -> H1 bf16 [128, 1024];
  L2: 4 matmuls, stationary = block-diag w2.T pairs -> P2D [128, 512]
  (half A cols 0:256, B 256:512; lane r: b=r//32, j=(r%32)//16, f=r%16);
  act2 Lrelu+b2 -> HJ, then DVE tensor_reduce -> ACC[:, p].
L2 and act2 are emitted one pair behind L1/act1 (software pipelining) so
the PE and Act engines never stall on same-pair dependencies.
o=128 (p=7, B, b=7, j=1): its L2 only covers i<128; cols 384:512 of lanes
112:128 are zero-filled, adding exactly 128*lrelu(b2) to ACC[112:128, 7]
(host subtracts).

Final combine on host: out = x1 + scale*(w3 . R + 32640*b3).
"""
import numpy as np

L, D = 256, 64
B_PER_CORE = 4
N_CORES = 8
NPAIRS = 32640  # 256*255/2

_CACHE = {}
import os as _os
N_BATCH = int(_os.environ.get("EPINN_BATCH", str(B_PER_CORE)))
N_RUN_CORES = int(_os.environ.get("EPINN_CORES", str(N_CORES)))


def _build_program():
    import concourse.bacc as bacc
    import concourse.mybir as mybir
    import concourse.tile as tile
    from concourse.bass import AP
    from contextlib import ExitStack

    f32 = mybir.dt.float32
    bf16 = mybir.dt.bfloat16
    u16 = mybir.dt.uint16
    AF = mybir.ActivationFunctionType
    ALU = mybir.AluOpType

    nc = bacc.Bacc("TRN2", target_bir_lowering=False, debug=False,
                   num_devices=N_CORES)

    # ---- DRAM parameters (per core) ----
    # embx[b] = [2, 64, 256]: [0] = X2T (bf16), [1] = X2T rolled by -64
    embx_d = nc.declare_dram_parameter("embx", [B_PER_CORE, 2, D, L], bf16,
                                       isOutput=False)
    # wall = [w1sa | w1sb | w2d4 | w2t] packed; ball = [b1s | b2s]
    wall_d = nc.declare_dram_parameter("wall", [128, 448], bf16, isOutput=False)
    ball_d = nc.declare_dram_parameter("ball", [128, 2], f32, isOutput=False)

    acc_o = nc.declare_dram_parameter("acc_o", [B_PER_CORE, 128, 8], f32,
                                      isOutput=True)

    with tile.TileContext(nc) as tc, ExitStack() as ctx:
        cpool = ctx.enter_context(tc.tile_pool(name="consts", bufs=1))
        ppool = ctx.enter_context(tc.tile_pool(name="persist", bufs=1))
        mpool = ctx.enter_context(tc.tile_pool(name="mbufs", bufs=3))
        hpool = ctx.enter_context(tc.tile_pool(name="hbufs", bufs=4))
        jpool = ctx.enter_context(tc.tile_pool(name="junk", bufs=2))
        pp1 = ctx.enter_context(tc.tile_pool(name="p1", bufs=3, space="PSUM"))
        pp2 = ctx.enter_context(tc.tile_pool(name="p2", bufs=2, space="PSUM"))

        # ---- static weights / consts ----
        WALL = cpool.tile([128, 448], bf16)
        BALL = cpool.tile([128, 2], f32)
        W1SA = WALL[:, 0:128]
        W1SB = WALL[:, 128:256]
        W2D4 = WALL[:, 256:384]
        W2T = WALL[:, 384:448]
        B1S = BALL[:, 0:1]
        B2S = BALL[:, 1:2]

        XUs = [None] * N_BATCH
        XSNs = [None] * N_BATCH
        ACCs = [None] * N_BATCH

        def emit_setup(b):
            XU = ppool.tile([128, 256], bf16, name=f"xu{b}")
            XSP = ppool.tile([128, 512], bf16, name=f"xsp{b}")
            eb = embx_d[b]  # [2, 64, 256]
            # XU: both halves = X2T -> in AP [2(stride 0), 64, 256]
            xu_src = AP(eb.tensor, eb.offset,
                        [[0, 2], [256, 64], [1, 256]])
            nc.sync.dma_start(XU[:], xu_src)
            # XSP: [(g d), (rep f)] <- in AP [2, 64, 2(stride 0), 256]
            xsp_src = AP(eb.tensor, eb.offset,
                         [[16384, 2], [256, 64], [0, 2], [1, 256]])
            nc.sync.dma_start(XSP[:].rearrange("p (r f) -> p r f", r=2),
                              xsp_src)
            ACC = ppool.tile([128, 8], f32, name=f"acc{b}")
            nc.gpsimd.memset(ACC[:], 0.0)
            XUs[b] = XU
            XSNs[b] = XSP
            ACCs[b] = ACC

        def emit_front(b, p):
            """sub/add/abs + L1 + act1 for pair (b, p); returns H1 tiles."""
            XU, XSP = XUs[b], XSNs[b]
            pstride = XU[:].ap.copy()[0][0]
            XU_B = AP(XU.tensor, XU.offset,
                      [[pstride, 128], [0, 8], [1, 256]])
            xsp_pstride = XSP[:].ap.copy()[0][0]

            o0 = 8 * p + 1
            # D2 block c: lanes 0:64 = x_i - x_{i+o0+c}, 64:128 = (o0+c+64)
            # S2 block c: same offsets, x_i + x_j
            D2 = mpool.tile([128, 8, 256], bf16, tag="d2")
            S2 = mpool.tile([128, 8, 256], bf16, tag="s2")
            XSP_W = AP(XSP.tensor, XSP.offset + o0,
                       [[xsp_pstride, 128], [1, 8], [1, 256]])
            nc.vector.tensor_tensor(out=D2[:], in0=XU_B, in1=XSP_W,
                                    op=ALU.subtract)
            nc.vector.tensor_tensor(out=S2[:], in0=XU_B, in1=XSP_W,
                                    op=ALU.add)
            nc.vector.tensor_scalar(
                out=D2[:].bitcast(u16), in0=D2[:].bitcast(u16),
                scalar1=0x7FFF, scalar2=None, op0=ALU.bitwise_and)

            H1s = []
            for h in (0, 1):  # half A/B
                P1 = pp1.tile([128, 1024], f32, tag="p1")
                for c2 in (0, 2):
                    nc.tensor.matmul(
                        P1[:, 256 * c2:256 * c2 + 512], W1SA[:],
                        D2[:, 4 * h + c2:4 * h + c2 + 2, :],
                        start=True, stop=False, skip_group_check=True)
                for c2 in (0, 2):
                    nc.tensor.matmul(
                        P1[:, 256 * c2:256 * c2 + 512], W1SB[:],
                        S2[:, 4 * h + c2:4 * h + c2 + 2, :],
                        start=False, stop=True, skip_group_check=True)
                H1 = hpool.tile([128, 1024], bf16, tag="h1")
                nc.scalar.activation(H1[:], P1[:], AF.Lrelu, bias=B1S[:],
                                     scale=1.0, alpha=0.01)
                H1s.append(H1)
            return H1s

        def emit_back(b, p, H1s):
            """L2 + act2 + accum for pair (b, p)."""
            ACC = ACCs[b]
            P2D = pp2.tile([128, 512], f32, tag="p2")
            for h in (0, 1):
                H1 = H1s[h]
                for bb in range(4):
                    if p == 7 and h == 1 and bb == 3:
                        nc.tensor.matmul(
                            P2D[96:128, 256:512], W2T[:, 0:32],
                            H1[:, 768:1024],
                            start=True, stop=False, skip_group_check=True,
                            tile_position=(0, 96))
                        nc.tensor.matmul(
                            P2D[96:128, 256:384], W2T[:, 32:64],
                            H1[:, 768:896],
                            start=False, stop=True, skip_group_check=True,
                            tile_position=(0, 96))
                    else:
                        nc.tensor.matmul(
                            P2D[32 * bb:32 * bb + 32, 256 * h:256 * h + 256],
                            W2D4[:, 32 * bb:32 * bb + 32],
                            H1[:, 256 * bb:256 * bb + 256],
                            start=True, stop=True, skip_group_check=True,
                            tile_position=(0, 32 * bb))

            HJ = jpool.tile([128, 512], bf16, tag="hj")
            nc.scalar.activation(HJ[:], P2D[:], AF.Lrelu, bias=B2S[:],
                                 scale=1.0, alpha=0.01,
                                 accum_out=ACC[:, p:p + 1])

        # ---- software-pipelined emission over the flat pair list ----
        emit_setup(0)
        nc.sync.dma_start(WALL[:], wall_d[:])
        nc.sync.dma_start(BALL[:], ball_d[:])
        pairs = [(b, p) for b in range(N_BATCH) for p in range(8)]
        prev = None
        for k, (b, p) in enumerate(pairs):
            H1s = emit_front(b, p)
            if prev is not None:
                pb, pp, pH = prev
                emit_back(pb, pp, pH)
                if pp == 7:
                    nc.sync.dma_start(acc_o[pb], ACCs[pb][:])
            prev = (b, p, H1s)
            if p == 0 and b + 1 < N_BATCH:
                emit_setup(b + 1)
        pb, pp, pH = prev
        emit_back(pb, pp, pH)
        nc.sync.dma_start(acc_o[pb], ACCs[pb][:])

    nc.compile()
    return nc


def _get_program():
    key = ("prog", N_BATCH)
    if key not in _CACHE:
        _CACHE[key] = _build_program()
    return _CACHE[key]


def _get_runner():
    """Build (once) a cached jitted SPMD executable for the program."""
    key = ("runner", N_BATCH, N_RUN_CORES)
    if key in _CACHE:
        return _CACHE[key]
    import jax
    import numpy as _np
    import concourse.mybir as mybir
    from jax.sharding import Mesh, PartitionSpec
    from jax.experimental.shard_map import shard_map
    from concourse import bass2jax
    from concourse.bass2jax import _bass_exec_p, partition_id_tensor

    bass2jax.install_neuronx_cc_hook()
    nc = _get_program()
    n_cores = N_RUN_CORES

    partition_name = (nc.partition_id_tensor.name
                      if nc.partition_id_tensor else None)
    in_names, out_names, out_avals, zero_shapes = [], [], [], []
    for alloc in nc.m.functions[0].allocations:
        if not isinstance(alloc, mybir.MemoryLocationSet):
            continue
        name = alloc.memorylocations[0].name
        if alloc.kind == "ExternalInput":
            if name != partition_name:
                in_names.append(name)
        elif alloc.kind == "ExternalOutput":
            out_names.append(name)
            shape = tuple(alloc.tensor_shape)
            dtype = mybir.dt.np(alloc.dtype)
            out_avals.append(jax.core.ShapedArray(shape, dtype))
            zero_shapes.append((shape, dtype))
    n_params = len(in_names)
    n_outs = len(out_avals)
    all_in_names = list(in_names) + list(out_names)
    if partition_name is not None:
        all_in_names.append(partition_name)
    donate = tuple(range(n_params, n_params + n_outs))

    def _body(*args):
        operands = list(args)
        if partition_name is not None:
            operands.append(partition_id_tensor())
        outs = _bass_exec_p.bind(
            *operands, out_avals=tuple(out_avals), in_names=tuple(all_in_names),
            out_names=tuple(out_names), lowering_input_output_aliases=(),
            sim_require_finite=True, sim_require_nnan=True, nc=nc)
        return tuple(outs)

    devices = jax.devices()[:n_cores]
    mesh = Mesh(_np.asarray(devices), ("core",))
    in_specs = (PartitionSpec("core"),) * (n_params + n_outs)
    out_specs = (PartitionSpec("core"),) * len(out_names)
    sharded = jax.jit(
        shard_map(_body, mesh=mesh, in_specs=in_specs, out_specs=out_specs,
                  check_rep=False),
        donate_argnums=donate, keep_unused=True)

    def run(in_maps):
        concat_in = [
            np.concatenate([np.asarray(in_maps[c][nm]) for c in range(n_cores)],
                           axis=0)
            for nm in in_names
        ]
        concat_zeros = [np.zeros((n_cores * s[0], *s[1:]), d)
                        for (s, d) in zero_shapes]
        out_arrs = sharded(*concat_in, *concat_zeros)
        return [
            {nm: np.asarray(out_arrs[i]).reshape(n_cores, *out_avals[i].shape)[c]
             for i, nm in enumerate(out_names)}
            for c in range(n_cores)
        ]

    _CACHE[key] = run
    return run


def _prep_inputs(emb, tw, w1, b1, w2, b2):
    import ml_dtypes
    bfl = ml_dtypes.bfloat16

    w1 = np.asarray(w1, np.float32)
    w1bt = w1[:, 64:].T               # [64, 64] abs-diff part
    w1at = 0.5 * w1[:, :64].T         # [64, 64] sum part
    w1sa = np.zeros((128, 128), np.float32)
    w1sa[0:64, 0:64] = w1bt
    w1sa[64:128, 64:128] = w1bt
    w1sb = np.zeros((128, 128), np.float32)
    w1sb[0:64, 0:64] = w1at
    w1sb[64:128, 64:128] = w1at
    w1sa = w1sa.astype(bfl)
    w1sb = w1sb.astype(bfl)

    w2f = np.asarray(w2, np.float32)
    w2d4 = np.zeros((128, 128), np.float32)
    for bb in range(4):
        w2d4[0:64, 32 * bb:32 * bb + 16] = w2f.T
        w2d4[64:128, 32 * bb + 16:32 * bb + 32] = w2f.T
    w2d4 = w2d4.astype(bfl)
    w2t2 = np.zeros((128, 64), np.float32)
    w2t2[0:64, 0:16] = w2f.T          # W2J0: j=0 slot, j=1 zero
    w2t2[64:128, 48:64] = w2f.T       # W2J1: j=1 slot, j=0 zero
    w2t2 = w2t2.astype(bfl)

    b1v = np.asarray(b1, np.float32)
    b2v = np.asarray(b2, np.float32)
    b1s = np.concatenate([b1v, b1v]).reshape(128, 1).astype(np.float32)
    b2s = np.tile(b2v, 8).reshape(128, 1).astype(np.float32)
    wall = np.concatenate([w1sa, w1sb, w2d4, w2t2], axis=1)  # [128, 448]
    ball = np.concatenate([b1s, b2s], axis=1)                # [128, 2]
    return {"wall": np.ascontiguousarray(wall),
            "ball": np.ascontiguousarray(ball)}


def _prep_embx(emb, tw):
    """[B, 2, 64, 256] bf16: [b,0] = (emb*tw).T, [b,1] = same rolled by -64."""
    import ml_dtypes
    bfl = ml_dtypes.bfloat16
    x = (emb[:, :-1] * tw[None, :-1]).reshape(-1, L, D)   # [B, 256, 64] f32
    xt = np.ascontiguousarray(x.transpose(0, 2, 1)).astype(bfl)
    xr = np.concatenate([xt[..., 64:], xt[..., :64]], axis=-1)
    return np.ascontiguousarray(np.stack([xt, xr], axis=1))


def kernel(emb, tw, tb, w1, b1, w2, b2, w3, b3, scale):
    run = _get_runner()

    emb = np.asarray(emb, np.float32)
    tw = np.asarray(tw, np.float32)

    shared = _prep_inputs(emb, tw, w1, b1, w2, b2)
    embx = _prep_embx(emb, tw)
    in_maps = []
    for c in range(N_CORES):
        m = dict(shared)
        m["embx"] = np.ascontiguousarray(
            embx[c * B_PER_CORE:(c + 1) * B_PER_CORE])
        in_maps.append(m)

    core_results = run(in_maps[:N_RUN_CORES])

    x1 = emb @ tw + float(tb[0])  # [32] f32 on host
    w3v = np.asarray(w3, np.float32)[0]
    b2v = np.asarray(b2, np.float32)
    # zero-filled block contributes 128*lrelu(b2) to lanes 112:128 col 7
    zero_corr = 128.0 * np.where(b2v > 0, b2v, 0.01 * b2v)
    out = np.zeros(32, np.float32)
    for c in range(N_RUN_CORES):
        acc = core_results[c]["acc_o"]   # [4, 128, 8]
        for b in range(N_BATCH):
            R = acc[b].reshape(8, 16, 8).sum(axis=(0, 2)) - zero_corr
            out[c * B_PER_CORE + b] = (
                x1[c * B_PER_CORE + b]
                + float(scale[0]) * (R @ w3v + float(b3[0]) * NPAIRS)
            )
    return out
